# revision 19
# baseline (speedup 1.0000x reference)
"""Bass/Trainium2 kernel for nn_CrossAttention_57526791963210.

Panorama cross-view attention: B=4 groups x T=26 views, each view's 256
tokens (C=128) attend to the tokens of its 3-4 neighbor views in a fixed
spherical adjacency graph, with 4 heads of dh=32, followed by an output
projection.

Sharding: 8 cores = 4 groups x 2 halves of the 26 views. The adjacency
graph has an automorphism sigma (antipodal map: 0<->1, ring1<->ring3
shifted by 4, ring2 rotated by 4) that maps one half onto the other, so a
single compiled program serves all 8 cores; the host permutes views by
sigma for odd cores.

The kernel is ScalarE(exp)-bound: 96 units x [128,1024] exp = ~12.6M
elements/core at 1 elem/cycle/lane.  Everything else is structured to
keep the ACT exp stream saturated:
  - x arrives as one [C, T, L] DRAM tensor with views pre-ordered by
    first use; a few chunked DMAs on the sync HWDGE ring start compute
    after ~2 us instead of serializing 26 per-view DIRECT2Ds (~23 us).
  - weights/biases ride the scalar HWDGE + gpsimd SWDGE rings in
    parallel with the x chunks.
  - PSUM: 3 rotating 2-bank score slots (pure scores), 1 bank for the
    per-dst [O | D] accumulators, 1 bank for small-matmul staging
    (K/V/Q projections and the out-proj), so projections never steal a
    score slot turn.
  - per (dst,src,head-pair) unit: scores S^T = K_h @ Q_h^T (row-packed,
    K=dh=32), exp on ScalarE (scale=1/sqrt(32) folded, scores bounded
    ~+-2.5 so no max pass), attnV + denominator matmuls col-packed into
    the [O | D] bank.
  - finish: reciprocal_approx_fast (single DVE op) + normalize,
    f32r out-proj, bias add, bf16 DMA out.
"""

import math

import numpy as np
import ml_dtypes

import concourse.bass as bass
import concourse.tile as tile
from concourse import mybir
from concourse.vector_clock import ScopedClock
from concourse.bass_utils import run_bass_kernel_spmd

C = 128
T = 26
L = 256
H = 4
DH = 32
B = 4
N_CORES = 8
SCALE = 1.0 / math.sqrt(DH)

BF16 = mybir.dt.bfloat16
F32 = mybir.dt.float32
F32R = mybir.dt.float32r


# ---------------------------------------------------------------- graph ----
def _build_graph():
    angles = [(0, 90), (0, -90)]
    for i in range(1, 4):
        angles += [(0 + 45 * j, -90 + 45 * i) for j in range(8)]
    angles = np.array(angles)
    d = np.abs(angles.reshape(26, 1, 2) - angles.reshape(1, 26, 2))
    d = np.sum(np.minimum(d, 360 - d), axis=2)
    ab = d == 45
    ab[0, -8::2] = True
    ab[-8::2, 0] = True
    ab[1, 2:10:2] = True
    ab[2:10:2, 1] = True
    return ab


_AB = _build_graph()
NBRS = [list(np.argwhere(_AB[i]).reshape(-1)) for i in range(26)]

# automorphism of the graph exchanging the two dst halves
SIGMA = np.zeros(26, np.int64)
SIGMA[0], SIGMA[1] = 1, 0
for _j in range(8):
    SIGMA[2 + _j] = 18 + (_j + 4) % 8
    SIGMA[18 + _j] = 2 + (_j + 4) % 8
    SIGMA[10 + _j] = 10 + (_j + 4) % 8

A_DST = [0, 2, 3, 4, 5, 6, 7, 8, 9, 10, 11, 12, 13]
N_DST = len(A_DST)

# views renumbered in first-use order (dst, then its nbrs, per dst): the
# host writes x in this order so the chunked input DMAs are contiguous
# and the first attention unit's views arrive in the first small chunk.
ORDER = []
for _d in A_DST:
    for _v in [_d] + NBRS[_d]:
        if _v not in ORDER:
            ORDER.append(_v)
for _v in range(T):
    if _v not in ORDER:
        ORDER.append(_v)
SLOT = {v: j for j, v in enumerate(ORDER)}
DST_P = [SLOT[d] for d in A_DST]
NBRS_P = [[SLOT[u] for u in NBRS[d]] for d in A_DST]
SRC_SET = sorted({u for nb in NBRS_P for u in nb})

# input DMA chunks (in slot order): a tiny first chunk so the first
# projection can start ~1 us in, then medium chunks.
_CH = [2, 4, 6, 7, 7]
assert sum(_CH) == T
CHUNKS = []
_o = 0
for _n in _CH:
    CHUNKS.append((_o, _n))
    _o += _n


# ------------------------------------------------- tile tail-drain patch ----
# Stock TileContext._drain_and_barrier attaches one sem wait per tracked
# proc to a single InstDrain; this walrus build caps Drain at 1 sync wait
# ("Too many sync wait commands").  Redistribute the extra waits onto
# dedicated NOPs (one wait each) right after the drain — same engine,
# serial order, so semantics are unchanged.
def _patched_drain_and_barrier(self, tick_clock, wait_clock):
    nc = self.nc
    drain_inst = nc.sync.drain()
    wait_clock.add_sem_waits(
        drain_inst.ins, ScopedClock({None: tick_clock.global_clock})
    )
    si = drain_inst.ins.sync_info
    if si is not None and len(si.on_wait) > 1:
        waits = list(si.on_wait)
        si.on_wait = waits[:1]
        for w in waits[1:]:
            nop = nc.sync.nop(nofuse=True)
            nop.ins.sync_info = mybir.SyncInfo(on_wait=[w], on_update=[])

    nc.all_engine_barrier()
    assert self.sems is not None
    popped = nc._tile_sem_poison_stack.pop()
    assert popped is self._sem_poison
    nc.clear_and_free_semaphores(list(self.sems.allocated().values()))
    nc.all_engine_barrier()


tile.TileContext._drain_and_barrier = _patched_drain_and_barrier


def _split_excess_waits(nc):
    """Walrus in this build accepts at most 1 sync wait per instruction
    (2 for EventSemaphore), but the Tile scheduler attaches as many as an
    instruction needs.  Hoist the excess onto EventSemaphore instructions
    inserted immediately before, on the same engine — same-engine program
    order makes this semantics-preserving."""
    n_new = 0
    for f in nc.m.functions:
        for bb in f.blocks:
            insts = bb.instructions
            new_list = []
            for inst in insts:
                si = inst.sync_info
                cap = 2 if isinstance(inst, mybir.InstEventSemaphore) else 1
                if si is not None and si.on_wait and len(si.on_wait) > cap:
                    waits = list(si.on_wait)
                    extra = waits[cap:]
                    si.on_wait = waits[:cap]
                    while extra:
                        chunk, extra = extra[:2], extra[2:]
                        ev = mybir.InstEventSemaphore(
                            name=f"waitsplit_{nc.next_id()}", ins=[], outs=[]
                        )
                        ev.engine = inst.engine
                        ev.sync_info = mybir.SyncInfo(on_wait=chunk, on_update=[])
                        new_list.append(ev)
                        n_new += 1
                new_list.append(inst)
            insts[:] = new_list
    return n_new


# ------------------------------------------------------------- program ----
def _build_program():
    nc = bass.Bass(trn_type="TRN2")

    xb = nc.dram_tensor("xb", [C, T, L], BF16, kind="ExternalInput")
    wqkvT = nc.dram_tensor("wqkvT", [C, 3 * C], BF16, kind="ExternalInput")
    woT = nc.dram_tensor("woT", [C, C], F32R, kind="ExternalInput")
    # [bk | bvb | bq | bo], each [C, L]
    bias = nc.dram_tensor("bias", [C, 4 * L], F32, kind="ExternalInput")
    yb = nc.dram_tensor("yb", [N_DST, C, L], BF16, kind="ExternalOutput")

    with tile.TileContext(nc) as tc:
        with (
            tc.tile_pool(name="const", bufs=1) as constp,
            tc.tile_pool(name="views", bufs=1) as views,
            tc.tile_pool(name="epool", bufs=8) as epool,
            tc.tile_pool(name="opool", bufs=2) as opool,
        ):
            # weights on the scalar HWDGE ring (ACT is idle until the
            # first exp); wo/bias via gpsimd SWDGE; x chunks on sync.
            w_t = constp.tile([C, 3 * C], BF16, tag="w")
            nc.scalar.dma_start(out=w_t, in_=wqkvT[:, :])
            bkv_t = constp.tile([C, 2 * L], F32, tag="bkv")
            nc.scalar.dma_start(out=bkv_t, in_=bias[:, 0:2 * L])
            bqo_t = constp.tile([C, 2 * L], F32, tag="bqo")
            nc.scalar.dma_start(out=bqo_t, in_=bias[:, 2 * L:4 * L])
            wo_t = constp.tile([C, C], F32R, tag="wo")
            nc.scalar.dma_start(out=wo_t, in_=woT[:, :])
            ones_t = constp.tile([C, DH], BF16, tag="ones")
            nc.vector.memset(ones_t, 1.0)

            xc = []
            for k, (off, n) in enumerate(CHUNKS):
                xt = views.tile([C, n, L], BF16, tag=f"xc{k}", name=f"xc{k}")
                nc.sync.dma_start(out=xt, in_=xb[:, off:off + n, :])
                xc.append(xt)

            def xv(v):
                for (off, n), xt in zip(CHUNKS, xc):
                    if off <= v < off + n:
                        return xt[:, v - off, :]
                raise AssertionError(v)

            _pipeline(nc, tc, views, epool, opool,
                      w_t, wo_t, bkv_t, bqo_t, ones_t, xv, yb)

    _split_excess_waits(nc)
    return nc


def _pipeline(nc, tc, views, epool, opool, w_t, wo_t, bkv_t, bqo_t, ones_t,
              xv, yb):
    with (
        tc.tile_pool(name="sps", bufs=3, space="PSUM") as sps,
        tc.tile_pool(name="accp", bufs=1, space="PSUM") as accp,
        tc.tile_pool(name="projp", bufs=1, space="PSUM") as projp,
    ):
        kvt = {}
        qt = {}

        def emit_proj_kv(v):
            pj = projp.tile([C, 2 * L], F32, tag="pj", name=f"kv{v}")
            nc.tensor.matmul(pj[:, 0:L], lhsT=w_t[:, C:2 * C], rhs=xv(v),
                             start=True, stop=True, skip_group_check=True)
            for c in range(2):
                nc.tensor.matmul(
                    pj[:, L + c * C:L + (c + 1) * C],
                    lhsT=xv(v)[:, c * C:(c + 1) * C],
                    rhs=w_t[:, 2 * C:3 * C],
                    start=True, stop=True, skip_group_check=True)
            kvt[v] = views.tile([C, 2 * L], BF16, tag=f"kv{v}", name=f"kvt{v}")
            nc.vector.tensor_add(kvt[v], pj, bkv_t)

        def emit_proj_q(v):
            # Q rides a score-slot turn so the projp chain stays short
            pj = sps.tile([C, H * L], F32, tag="s", name=f"qp{v}")
            nc.tensor.matmul(pj[:, 0:L], lhsT=w_t[:, 0:C], rhs=xv(v),
                             start=True, stop=True, skip_group_check=True)
            qt[v] = views.tile([C, L], BF16, tag=f"q{v}", name=f"qt{v}")
            nc.vector.tensor_add(qt[v], pj[:, 0:L], bqo_t[:, 0:L])

        def emit_proj(p):
            kind, v = p
            if kind == "kv":
                emit_proj_kv(v)
            else:
                emit_proj_q(v)

        dst_state = {}

        def emit_scores_exp(u):
            i, dst, si_, src, pair = u
            sp = sps.tile([C, H * L], F32, tag="s", name=f"sp{i}_{si_}_{pair}")
            for c in range(2):
                for hl in range(2):
                    h = 2 * pair + hl
                    nc.tensor.matmul(
                        sp[:, hl * 2 * L + c * L:hl * 2 * L + (c + 1) * L],
                        lhsT=kvt[src][h * DH:(h + 1) * DH, c * C:(c + 1) * C],
                        rhs=qt[dst][h * DH:(h + 1) * DH, :],
                        start=True,
                        stop=True,
                        tile_position=(h * DH, 0),
                    )
            et = epool.tile([C, H * L], BF16, tag="e", name=f"et{i}_{si_}_{pair}")
            nc.scalar.activation(
                et, sp, mybir.ActivationFunctionType.Exp, scale=SCALE
            )
            return et

        def emit_attnv_d(u, et):
            i, dst, si_, src, pair = u
            if i not in dst_state:
                acc = accp.tile([C, 2 * L], F32, tag="acc", name=f"acc{i}")
                dst_state[i] = [acc, False]
            st = dst_state[i]
            acc = st[0]
            o_ps = acc[:, 0:L]
            d_ps = acc[:, L:2 * L]
            # a matmul's start=True clears has_written for its OUTPUT
            # PARTITION ROWS across the whole bank width.  o and d share
            # the bank at the same rows (different free offsets), so per
            # dst only the first attnV write of each head's row group
            # carries start=True — it also clears the d region's stale
            # bits (d's first write then lands as an overwrite).  attnV
            # pair before d pair keeps that order and lets the two
            # col-groups of each step run concurrently in the PE array.
            last = si_ == len(NBRS_P[i]) - 1
            for c in range(2):
                for kind in range(2):
                    for hl in range(2):
                        h = 2 * pair + hl
                        esec = et[:, hl * 2 * L + c * L:hl * 2 * L + (c + 1) * L]
                        tgt = o_ps if kind == 0 else d_ps
                        lhsT = (
                            kvt[src][:, L + c * C + h * DH:L + c * C + (h + 1) * DH]
                            if kind == 0 else ones_t
                        )
                        nc.tensor.matmul(
                            tgt[h * DH:(h + 1) * DH, :],
                            lhsT=lhsT,
                            rhs=esec,
                            start=si_ == 0 and c == 0 and kind == 0,
                            stop=last and c == 1,
                            tile_position=(0, h * DH),
                            skip_group_check=True,
                        )

        fin_state = {}

        def emit_finish_dve(u):
            # DVE-only half of the finish: evict [O | D] so the single acc
            # bank frees fast, then normalize.  No PE instruction here, so
            # nothing head-of-line blocks the tensor queue.
            i, dst, si_, src, pair = u
            acc = dst_state.pop(i)[0]
            od = opool.tile([C, 2 * L], F32, tag="od", name=f"od{i}")
            nc.vector.tensor_copy(out=od, in_=acc)
            dr = opool.tile([C, L], F32, tag="dr", name=f"dr{i}")
            nc.vector.reciprocal(dr, od[:, L:2 * L])
            on = opool.tile([C, L], F32R, tag="on", name=f"on{i}")
            nc.vector.tensor_mul(on, od[:, 0:L], dr)
            fin_state[i] = on

        def emit_finish_out(i):
            # PE half, deferred a few units so `on` is ready before the
            # yp matmul reaches the head of the in-order PE queue.
            on = fin_state.pop(i)
            yp = projp.tile([C, 2 * L], F32, tag="pj", name=f"yp{i}")
            nc.tensor.matmul(
                yp[:, 0:L], lhsT=wo_t, rhs=on, start=True, stop=True,
                skip_group_check=True,
            )
            ys = opool.tile([C, L], BF16, tag="ys", name=f"ys{i}")
            nc.vector.tensor_add(ys, yp[:, 0:L], bqo_t[:, L:2 * L])
            nc.sync.dma_start(out=yb[i, :, :], in_=ys)

        # global projection FIFO in first-need order, with dedup; a dst's
        # q rides a score slot (parallel to the projp chain) so it goes
        # ahead of that dst's kv turns.
        proj_fifo = []
        seen = set()
        for i, dst in enumerate(DST_P):
            if ("q", dst) not in seen:
                seen.add(("q", dst))
                proj_fifo.append(("q", dst))
            for src in NBRS_P[i]:
                if ("kv", src) not in seen:
                    seen.add(("kv", src))
                    proj_fifo.append(("kv", src))

        emitted = set()

        def ensure_proj(p):
            if p not in emitted:
                emitted.add(p)
                emit_proj(p)

        def drain_one_ahead():
            while proj_fifo and proj_fifo[0] in emitted:
                proj_fifo.pop(0)
            if proj_fifo:
                ensure_proj(proj_fifo.pop(0))

        units = []
        for i, dst in enumerate(DST_P):
            for si_, src in enumerate(NBRS_P[i]):
                for pair in range(2):
                    units.append((i, dst, si_, src, pair))

        # per unit: required projections, then scores+exp; attnV/d is
        # emitted TWO units late so a slow (HAM-cold) attnV block never
        # sits between the exp stream and its next scores — the scores
        # for exp u+1 and u+2 are already in their slots when exp u ends.
        # One look-ahead projection every other unit keeps dst-boundary
        # projection bursts off the critical path.
        LAG = 3
        FIN_DEFER = 4
        pending = []
        fin_defer = []

        def pop_pending():
            pu, pet = pending.pop(0)
            emit_attnv_d(pu, pet)
            if pu[2] == len(NBRS_P[pu[0]]) - 1 and pu[4] == 1:
                emit_finish_dve(pu)
                fin_defer.append([pu[0], 0])

        for j, u in enumerate(units):
            i, dst, si_, src, pair = u
            ensure_proj(("kv", src))
            ensure_proj(("q", dst))
            et = emit_scores_exp(u)
            pending.append((u, et))
            if len(pending) > LAG:
                pop_pending()
            for e in fin_defer:
                e[1] += 1
            if fin_defer and fin_defer[0][1] >= FIN_DEFER:
                emit_finish_out(fin_defer.pop(0)[0])
            drain_one_ahead()
        # tail flush: interleave the remaining attnV pops with deferred
        # finish-outs so each yp matmul's normalize chain resolves while
        # the next attnV block runs, instead of serializing at the end.
        while pending:
            pop_pending()
            if fin_defer:
                emit_finish_out(fin_defer.pop(0)[0])
        while fin_defer:
            emit_finish_out(fin_defer.pop(0)[0])


_PROGRAM = {}


def _get_program():
    if "p" not in _PROGRAM:
        _PROGRAM["p"] = _build_program()
    return _PROGRAM["p"]


# --------------------------------------------------------------- kernel ----
def kernel(x, w_in, b_in, w_out, b_out, _trace=False):
    x = np.ascontiguousarray(np.asarray(x), dtype=np.float32)
    w_in = np.asarray(w_in, dtype=np.float32)
    b_in = np.asarray(b_in, dtype=np.float32)
    w_out = np.asarray(w_out, dtype=np.float32)
    b_out = np.asarray(b_out, dtype=np.float32)

    nc = _get_program()

    xr = x.reshape(B, T, C, L)
    bf = ml_dtypes.bfloat16
    bias_cat = np.concatenate(
        [
            np.repeat(b_in[C:2 * C].reshape(C, 1), L, axis=1),           # bk
            np.tile(b_in[2 * C:3 * C], 2)[None, :].repeat(C, axis=0),    # bvb
            np.repeat(b_in[0:C].reshape(C, 1), L, axis=1),               # bq
            np.repeat(b_out.reshape(C, 1), L, axis=1),                   # bo
        ],
        axis=1,
    )
    shared = {
        "wqkvT": np.ascontiguousarray(
            np.concatenate(
                [w_in[0:C].T, w_in[C:2 * C].T, w_in[2 * C:3 * C].T], axis=1
            )
        ).astype(bf),
        "woT": np.ascontiguousarray(w_out.T).astype(np.float32),
        "bias": np.ascontiguousarray(bias_cat),
    }

    ident = np.arange(T)
    order_arr = np.array(ORDER)
    in_maps = []
    perms = []
    for core in range(N_CORES):
        b = core // 2
        perm = SIGMA if core % 2 else ident
        perms.append(perm)
        # program slot j holds canonical view ORDER[j], physically
        # perm[ORDER[j]] of this core's group; layout [C, T, L]
        xb_c = np.ascontiguousarray(
            xr[b][perm[order_arr]].transpose(1, 0, 2)
        ).astype(bf)
        in_maps.append({"xb": xb_c, **shared})

    res = run_bass_kernel_spmd(
        nc, in_maps, core_ids=list(range(N_CORES)), trace=_trace
    )

    y = np.zeros((B, T, C, L), dtype=np.float32)
    for core in range(N_CORES):
        out_c = res.results[core]["yb"]
        b = core // 2
        perm = perms[core]
        for i, dorig in enumerate(A_DST):
            y[b, perm[dorig]] = out_c[i].astype(np.float32)

    out = y.reshape(B * T, C, 16, 16)
    if _trace:
        return out, res
    return out
